# revision 19
# baseline (speedup 1.0000x reference)
"""Row softmax on 8 TRN2 NeuronCores (Bass/Tile, SPMD data-parallel).

The reference computes softmax(x - rowmean(x)) per row, with exp approximated
by a 100-term Taylor series of 2**frac plus exp2 of the integer part.  That is
mathematically softmax(x, axis=1) (softmax is shift invariant; the Taylor
series converges to f32 precision), so the kernel computes a plain row softmax
with the ScalarEngine's Exp activation.

Sharding: pure data parallel — 262144 rows split into 8 shards of 32768 rows,
one per core; each row's reduction is local to its core.

Per-core layout: the 32768-row shard is viewed as [128 partitions, 256 rows,
256 cols]; partition p owns 256 consecutive rows, so every DMA moves large
per-partition-contiguous chunks (tiles of 8 rows/partition = 1 MiB per DMA,
8 KiB contiguous per partition).  Inputs stream on the sync HWDGE queue,
outputs on the gpsimd SWDGE queue, with 8-deep tile pools, so the two DMA
streams run back-to-back at the HBM rate for the whole kernel.
"""

import numpy as np

N, C = 262144, 256
N_CORES = 8
ROWS_PER_CORE = N // N_CORES        # 32768
P = 128                             # SBUF partitions
ROWS_PER_PART = ROWS_PER_CORE // P  # 256 rows owned by each partition
R = 8                               # rows per partition per tile
N_TILES = ROWS_PER_PART // R        # tiles per core

_NC_CACHE = {}


def _get_nc():
    if "nc" in _NC_CACHE:
        return _NC_CACHE["nc"]
    from contextlib import ExitStack

    import concourse.tile as tile
    from concourse import bacc, mybir

    nc = bacc.Bacc(
        "TRN2", target_bir_lowering=False, debug=False,
        enable_asserts=False, num_devices=N_CORES,
    )
    x_h = nc.dram_tensor("x", [ROWS_PER_CORE, C], mybir.dt.float32,
                         kind="ExternalInput")
    o_h = nc.dram_tensor("out", [ROWS_PER_CORE, C], mybir.dt.float32,
                         kind="ExternalOutput")
    x_prc = x_h.ap().rearrange("(p r) c -> p r c", p=P)
    o_prc = o_h.ap().rearrange("(p r) c -> p r c", p=P)

    with tile.TileContext(nc) as tc, ExitStack() as ctx:
        xp = ctx.enter_context(tc.tile_pool(name="xp", bufs=8))
        ep = ctx.enter_context(tc.tile_pool(name="ep", bufs=8))
        sp = ctx.enter_context(tc.tile_pool(name="sp", bufs=8))
        cp = ctx.enter_context(tc.tile_pool(name="cp", bufs=1))
        # memset zero-bias tile: a float bias would be materialized as a
        # const tensor whose DMA load serializes ahead of the first input
        # DMA on the HWDGE queue (~6us of kernel-start latency).
        bias0 = cp.tile([P, 1], mybir.dt.float32)
        nc.vector.memset(bias0[:], 0.0)
        for t in range(N_TILES):
            xt = xp.tile([P, R * C], mybir.dt.float32)
            # inputs issue from sync (HWDGE); outputs from gpsimd (SWDGE)
            # so input DMAs never queue behind output DMAs waiting on the
            # DVE mul chain (HWDGE is FIFO per issuing engine).
            nc.sync.dma_start(
                xt[:].rearrange("p (r c) -> p r c", c=C),
                x_prc[:, t * R:(t + 1) * R, :],
            )
            et = ep.tile([P, R * C], mybir.dt.float32)
            st = sp.tile([P, R], mybir.dt.float32)
            e3 = et[:].rearrange("p (r c) -> p r c", c=C)
            # Row sums come from either the ScalarEngine (per-row exp with
            # accum_out; ~6.2us/tile on ACT, nothing on DVE) or the
            # VectorEngine (one big exp ~2.4us on ACT + 2.7us reduce on
            # DVE).  Mixing the two 14:18 lands ACT and DVE at ~130us each;
            # balanced engines keep the output-drain cadence above the DMA
            # rate at the tail of the kernel once the input stream finishes.
            if t % 16 in (1, 3, 5, 8, 10, 12, 14):
                for r in range(R):
                    nc.scalar.activation(
                        et[:, r * C:(r + 1) * C],
                        xt[:, r * C:(r + 1) * C],
                        mybir.ActivationFunctionType.Exp,
                        bias=bias0[:],
                        accum_out=st[:, r:r + 1],
                    )
            else:
                nc.scalar.activation(
                    et[:], xt[:], mybir.ActivationFunctionType.Exp,
                    bias=bias0[:],
                )
                nc.vector.reduce_sum(st[:], e3, axis=mybir.AxisListType.X)
            rt = sp.tile([P, R], mybir.dt.float32)
            nc.vector.reciprocal(rt[:], st[:])
            nc.vector.tensor_mul(
                e3, e3, rt[:][:, :, None].broadcast_to((P, R, C))
            )
            nc.gpsimd.dma_start(o_prc[:, t * R:(t + 1) * R, :], e3)
    nc.compile()
    _NC_CACHE["nc"] = nc
    return nc


def _install_ntff_hook():
    """Make the optional antenv.axon_hooks module available so the
    trace=True / BASS_TRACE path of run_bass_kernel_spmd works under axon
    (the image's antenv package lacks axon_hooks; boot() skips the NTFF
    hook registration silently in that case)."""
    import sys
    import types

    try:
        import antenv.axon_hooks  # noqa: F401
    except ImportError:
        try:
            import antenv
        except ImportError:
            return
        mod = types.ModuleType("antenv.axon_hooks")
        holder = {}
        mod.set_axon_ntff_profile_hook = lambda h: holder.__setitem__("h", h)
        mod.get_axon_ntff_profile_hook = lambda: holder.get("h")
        sys.modules["antenv.axon_hooks"] = mod
        antenv.axon_hooks = mod
    from antenv.axon_hooks import (
        get_axon_ntff_profile_hook,
        set_axon_ntff_profile_hook,
    )

    if get_axon_ntff_profile_hook() is None:
        try:
            from trn_agent_boot.trn_boot import _ntff_profile_via_ctypes

            set_axon_ntff_profile_hook(
                _ntff_profile_via_ctypes("/opt/axon/libaxon_pjrt.so")
            )
        except Exception:
            pass


def _build_per_device_runner(nc):
    """Per-device dispatch in HBM-domain-interleaved order.

    Host->device uploads serialize, so with a single global dispatch each
    even core's NEFF executes exactly while its HBM-domain partner's
    (core+1) input upload streams into the same HBM stack, costing ~25us
    (measured 198-204us on even cores vs 174.7us on odd cores).
    Dispatching per-device executions in order 0,2,4,6,1,3,5,7 makes the
    upload that overlaps core i's execution always target a different HBM
    domain: all 8 cores then run at ~174.6us.
    """
    import jax
    import jax.numpy as jnp
    from concourse import bass2jax, mybir

    bass2jax.install_neuronx_cc_hook()

    partition_name = (
        nc.partition_id_tensor.name if nc.partition_id_tensor else None
    )
    in_names, out_names, out_avals = [], [], []
    for alloc in nc.m.functions[0].allocations:
        if not isinstance(alloc, mybir.MemoryLocationSet):
            continue
        assert alloc.memorylocations
        name = alloc.memorylocations[0].name
        if alloc.kind == "ExternalInput":
            if name != partition_name:
                in_names.append(name)
        elif alloc.kind == "ExternalOutput":
            assert alloc.tensor_shape is not None and alloc.dtype is not None
            out_names.append(name)
            out_avals.append(
                jax.core.ShapedArray(
                    tuple(alloc.tensor_shape), mybir.dt.np(alloc.dtype)
                )
            )
    n_params = len(in_names)
    all_in_names = tuple(in_names) + tuple(out_names)
    if partition_name is not None:
        # supplied as the last operand via PartitionIdOp, mirroring
        # run_bass_via_pjrt; this program never reads it (no collectives).
        all_in_names = all_in_names + (partition_name,)

    def _body(*args):
        operands = list(args)
        if partition_name is not None:
            operands.append(bass2jax.partition_id_tensor())
        outs = bass2jax._bass_exec_p.bind(
            *operands,
            out_avals=tuple(out_avals),
            in_names=all_in_names,
            out_names=tuple(out_names),
            lowering_input_output_aliases=(),
            sim_require_finite=True,
            sim_require_nnan=True,
            nc=nc,
        )
        return tuple(outs)

    donate = tuple(range(n_params, n_params + len(out_names)))
    jitted = jax.jit(_body, donate_argnums=donate, keep_unused=True)

    devs = jax.devices()[:N_CORES]
    zeros_makers = {
        d: jax.jit(
            lambda: tuple(jnp.zeros(a.shape, a.dtype) for a in out_avals),
            out_shardings=jax.sharding.SingleDeviceSharding(devs[d]),
        )
        for d in range(N_CORES)
    }

    def run(in_maps, order=(0, 2, 4, 6, 1, 3, 5, 7)):
        futures = {}
        for d in order:
            args = [
                jax.device_put(np.asarray(in_maps[d][n]), devs[d])
                for n in in_names
            ]
            zeros = zeros_makers[d]()  # created on-device: no H2D traffic
            futures[d] = jitted(*args, *zeros)
        return [
            {n: np.asarray(futures[d][i]) for i, n in enumerate(out_names)}
            for d in range(len(in_maps))
        ]

    return run


def _run(x, **spmd_kwargs):
    _install_ntff_hook()
    nc = _get_nc()
    x = np.ascontiguousarray(np.asarray(x), dtype=np.float32)
    assert x.shape == (N, C), x.shape
    shards = np.split(x, N_CORES, axis=0)
    in_maps = [{"x": np.ascontiguousarray(s)} for s in shards]

    if not spmd_kwargs:
        try:
            if "runner" not in _NC_CACHE:
                _NC_CACHE["runner"] = _build_per_device_runner(nc)
            results = _NC_CACHE["runner"](in_maps)
            out = np.concatenate([r["out"] for r in results], axis=0)
            return out, None
        except Exception:
            pass  # fall back to the stock global-dispatch path

    from concourse.bass_utils import run_bass_kernel_spmd

    res = run_bass_kernel_spmd(
        nc, in_maps, core_ids=list(range(N_CORES)), **spmd_kwargs
    )
    out = np.concatenate(
        [np.asarray(res.results[i]["out"]) for i in range(N_CORES)], axis=0
    )
    return out, res


def kernel(x):
    return _run(x)[0]
